# revision 5
# baseline (speedup 1.0000x reference)
"""Sparse regional cross-attention on 8 Trainium2 NeuronCores.

Reference computation (B=1, S=56320, H=8, D=64, P=128, R=2):
  - per-region binary masks over the latent sequence select which KV segments
    each query may attend to (global prompt + R regional prompts, 128 keys
    each); regional pass = softmax over the union of allowed segments
  - base pass: plain softmax attention over the global prompt
  - out = 0.5 * regional + 0.5 * base

Kernel strategy (v2):
  - sequence-parallel across 8 cores (7040 queries/core); queries host-sorted
    into 5 tile categories: global-only (segs {0}), region-1-only ({0,1}),
    region-2-only ({0,2}), both-regions ({0,1,2}, a0=0), mixed leftovers
    ({0,1,2} with per-chunk a0 data). Output un-permuted on host.
  - per (head, 512-query tile): K=66 bf16 score matmuls (qT + 2 bias rows;
    measured: K=66 streams at full rate), ONE fused exp per head over all
    segments, chunked PV matmuls (N=65, FWL-bound ~55ns) into a combined
    [128, 2, 512] PSUM tile: T0-half (base, vp has 2.0 denom col) and
    T12-half (regional segs accumulated); chunk c at offset 65c avoids bank
    crossings.
  - merge per head: ONE batched reciprocal [128,2,4] over both denominator
    sets + ONE fused tensor_mul over [128,2,4,64] numerators with the
    reciprocal broadcast; per tile a single gpsimd add folds the two halves
    into the output slab. The 0.5 blend is baked into vp's 2.0 denominator
    column (global tiles use a 1.0 column).
"""

import sys

for _p in ("/opt/trn_rl_repo",):
    if _p not in sys.path:
        sys.path.insert(0, _p)

import numpy as np
import ml_dtypes

# ---------------------------------------------------------------- constants
B, S, H, D, P, R = 1, 56320, 8, 64, 128, 2
N_CORES = 8
SSH = S // N_CORES          # 7040 queries per core
W_TILE = 512                # queries per tile
N_TILES = (SSH + W_TILE - 1) // W_TILE   # 14 (13 full + 1x384)
KQ = 66                     # q rows: 64 dims + 2 bias rows
LAT_T, LAT_H, LAT_W = 16, 44, 80
SCALE = D ** -0.5
NEG_BIAS = -30.0

_COMPILED = {}


# ------------------------------------------------------------ mask pipeline
def _resize_trilinear_np(m, tgt_shape):
    """numpy replica of jax.image.resize(..., 'trilinear', antialias=False)."""
    Bn, C, T, Hh, Ww = m.shape
    _, _, tT, tH, tW = tgt_shape
    out = m.astype(np.float32)

    def lin_weights(n_in, n_out):
        j = np.arange(n_out, dtype=np.float64)
        x = (j + 0.5) * (n_in / n_out) - 0.5
        lo = np.floor(x).astype(np.int64)
        frac = (x - lo).astype(np.float32)
        lo0 = np.clip(lo, 0, n_in - 1)
        lo1 = np.clip(lo + 1, 0, n_in - 1)
        Wm = np.zeros((n_out, n_in), np.float32)
        Wm[np.arange(n_out), lo0] += 1.0 - frac
        Wm[np.arange(n_out), lo1] += frac
        return Wm

    out = np.einsum('oi,bcihw->bcohw', lin_weights(T, tT), out)
    out = np.einsum('oi,bctiw->bctow', lin_weights(Hh, tH), out)
    out = np.einsum('oi,bcthi->bctho', lin_weights(Ww, tW), out)
    return out.astype(np.float32)


def _preprocess_mask_np(mask):
    m = np.transpose(mask, (3, 0, 1, 2))[:, None]  # [B,1,T,H,W]
    Bn = m.shape[0]
    T = m.shape[2]
    tgt = (Bn, 1, 1, LAT_H, LAT_W)
    pieces = [_resize_trilinear_np(m[:, :, :1], tgt)]
    for wi in range(1, T, 8):
        pieces.append(_resize_trilinear_np(m[:, :, wi:wi + 8], tgt))
    mm = np.concatenate(pieces, axis=2)[:, 0]
    return (mm > 0.5).astype(np.float32).reshape(Bn, -1)


def _preprocess_masks(region_masks):
    """region_masks [R, T, MH, MW, B] -> a0, a1, a2 each [S] float32 {0,1}."""
    try:
        import jax
        import jax.numpy as jnp

        cpu = jax.devices('cpu')[0]
        with jax.default_device(cpu):
            def one(mask):
                m = jnp.transpose(jnp.asarray(mask), (3, 0, 1, 2))[:, None]
                Bn, _, T, _, _ = m.shape
                tgt = (Bn, 1, 1, LAT_H, LAT_W)
                pieces = [jax.image.resize(m[:, :, :1], tgt, 'trilinear',
                                           antialias=False)]
                for wi in range(1, T, 8):
                    pieces.append(jax.image.resize(m[:, :, wi:wi + 8], tgt,
                                                   'trilinear',
                                                   antialias=False))
                mm = jnp.concatenate(pieces, axis=2)[:, 0]
                return (mm > 0.5).astype(jnp.float32).reshape(Bn, -1)

            masks = np.stack([np.asarray(one(region_masks[i]))
                              for i in range(region_masks.shape[0])], axis=0)
    except Exception:
        masks = np.stack([_preprocess_mask_np(region_masks[i])
                          for i in range(region_masks.shape[0])], axis=0)
    a1 = masks[0, 0]
    a2 = masks[1, 0]
    a0 = ((masks[0, 0] + masks[1, 0]) == 0).astype(np.float32)
    return a0, a1, a2


# ------------------------------------------------------------- bass kernel
def _build_kernel(cfg):
    """cfg = (t_g, t_r1, t_r2, t_b, kinds): leading tile counts per core for
    the global-only / region-1-only / region-2-only / both-regions
    categories; the rest are mixed 3-segment tiles whose per-chunk kinds
    ('g'/'n'/'x') are compile-time constants."""
    import concourse.bass as bass
    import concourse.tile as tile
    from concourse import bacc, mybir

    f32 = mybir.dt.float32
    bf16 = mybir.dt.bfloat16
    Exp = mybir.ActivationFunctionType.Exp

    t_g, t_r1, t_r2, t_b, kinds = cfg

    nc = bacc.Bacc("TRN2", target_bir_lowering=False, debug=False,
                   num_devices=N_CORES)

    qt_d = nc.dram_tensor("qt", [KQ, N_TILES, H, W_TILE], bf16,
                          kind="ExternalInput").ap()
    kt_d = nc.dram_tensor("kt", [KQ, 3, H, P], bf16,
                          kind="ExternalInput").ap()
    vp_d = nc.dram_tensor("vp", [128, 4, H, 65], bf16,
                          kind="ExternalInput").ap()
    am_d = nc.dram_tensor("am", [128, N_TILES, 4], f32,
                          kind="ExternalInput").ap()
    out_d = nc.dram_tensor("out", [SSH, H * D], bf16,
                           kind="ExternalOutput").ap()

    with tile.TileContext(nc) as tc:
        with (
            tc.tile_pool(name="singles", bufs=1) as singles,
            tc.tile_pool(name="qt", bufs=2) as qt_pool,
            tc.tile_pool(name="epool", bufs=3) as e_pool,
            tc.tile_pool(name="small", bufs=16) as sm_pool,
            tc.tile_pool(name="tall", bufs=2) as tall_pool,
            tc.tile_pool(name="slab", bufs=2) as slab_pool,
        ):
            kt_sb = singles.tile([KQ, 3, H, P], bf16)
            nc.sync.dma_start(out=kt_sb, in_=kt_d)
            vp_sb = singles.tile([128, 4, H, 65], bf16)
            nc.sync.dma_start(out=vp_sb, in_=vp_d)
            am_sb = singles.tile([128, N_TILES, 4], f32)
            nc.sync.dma_start(out=am_sb, in_=am_d)

            def tile_prologue(t):
                Wq = min(W_TILE, SSH - t * W_TILE)
                nch = Wq // 128
                qt_t = qt_pool.tile([KQ, H, W_TILE], bf16)
                nc.sync.dma_start(out=qt_t, in_=qt_d[:, t])
                return Wq, nch, qt_t

            def tile_epilogue(t, Wq, nch, slab):
                s0 = t * W_TILE
                nc.sync.dma_start(
                    out=out_d[s0:s0 + Wq, :].rearrange("(c p) f -> p c f",
                                                       p=128),
                    in_=slab[:, :nch, :])

            def t_views(T, nch):
                """numerator [128,2,nch,64] and denominator [128,2,nch]
                views of a combined [128, 2, 512] PSUM tile with chunk c
                at offset 65c."""
                vv = T[:, :, :260].rearrange("p w (c x) -> p w c x", x=65)
                return vv[:, :, :nch, 0:64], vv[:, :, :nch, 64]

            # ---- phase G: global-only tiles (segment 0; vp slot 3) ----
            with (
                tc.tile_pool(name="gsc", bufs=2, space="PSUM") as gsc,
                tc.tile_pool(name="gt0", bufs=2, space="PSUM") as gt0,
            ):
                for t in range(t_g):
                    Wq, nch, qt_t = tile_prologue(t)
                    slab = slab_pool.tile([128, 4, H * D], bf16)

                    def g_scores(h):
                        sc = gsc.tile([128, W_TILE], f32, tag="gs")
                        nc.tensor.matmul(
                            sc[:, :Wq], lhsT=kt_sb[:, 0, h, :],
                            rhs=qt_t[:, h, :Wq], start=True, stop=True)
                        return sc

                    sc_h = g_scores(0)
                    for h in range(H):
                        e = e_pool.tile([128, W_TILE], bf16, tag="e1")
                        nc.scalar.activation(e[:, :Wq], sc_h[:, :Wq], Exp)
                        if h + 1 < H:
                            sc_h = g_scores(h + 1)
                        T = gt0.tile([128, 4, 65], f32, tag="T")
                        for c in range(nch):
                            cs = slice(c * 128, (c + 1) * 128)
                            nc.tensor.matmul(T[:, c, :], lhsT=e[:, cs],
                                             rhs=vp_sb[:, 3, h, :],
                                             start=True, stop=True)
                        ri = sm_pool.tile([128, 4], f32, tag="ri")
                        nc.vector.reciprocal(ri[:, :nch], T[:, :nch, 64])
                        nc.vector.tensor_mul(
                            slab[:, :nch, h * 64:(h + 1) * 64],
                            T[:, :nch, 0:64],
                            ri[:, :nch, None].broadcast_to([128, nch, 64]))
                    tile_epilogue(t, Wq, nch, slab)

            # ---- phase R: single-region tiles (segs {0, r}) ----
            with (
                tc.tile_pool(name="rsc", bufs=2, space="PSUM") as rsc,
                tc.tile_pool(name="rT", bufs=2, space="PSUM") as rT,
            ):
                for t in range(t_g, t_g + t_r1 + t_r2):
                    rseg = 1 if t < t_g + t_r1 else 2
                    Wq, nch, qt_t = tile_prologue(t)
                    slab = slab_pool.tile([128, 4, H * D], bf16)
                    t_all = tall_pool.tile([128, 2, 4, H, 64], bf16)

                    def r_scores(h):
                        sc = rsc.tile([128, 2, W_TILE], f32, tag="rs")
                        for j, r in enumerate((0, rseg)):
                            nc.tensor.matmul(
                                sc[:, j, :Wq], lhsT=kt_sb[:, r, h, :],
                                rhs=qt_t[:, h, :Wq], start=True, stop=True)
                        return sc

                    sc_h = r_scores(0)
                    for h in range(H):
                        e = e_pool.tile([128, 2, W_TILE], bf16, tag="e2")
                        nc.scalar.activation(e[:, :, :Wq], sc_h[:, :, :Wq],
                                             Exp)
                        if h + 1 < H:
                            sc_h = r_scores(h + 1)
                        T = rT.tile([128, 2, W_TILE], f32, tag="T")
                        for c in range(nch):
                            cs = slice(c * 128, (c + 1) * 128)
                            o = slice(65 * c, 65 * c + 65)
                            nc.tensor.matmul(T[:, 0, o], lhsT=e[:, 0, cs],
                                             rhs=vp_sb[:, 0, h, :],
                                             start=True, stop=True)
                            nc.tensor.matmul(T[:, 1, o], lhsT=e[:, 1, cs],
                                             rhs=vp_sb[:, rseg, h, :],
                                             start=True, stop=True)
                        tn, td = t_views(T, nch)
                        ri = sm_pool.tile([128, 2, 4], f32, tag="ri2")
                        nc.vector.reciprocal(ri[:, :, :nch], td)
                        nc.vector.tensor_mul(
                            t_all[:, :, :nch, h, :], tn,
                            ri[:, :, :nch, None].broadcast_to(
                                [128, 2, nch, 64]))
                    nc.gpsimd.tensor_add(
                        slab[:, :nch, :],
                        t_all[:, 0, :nch].rearrange("p c h d -> p c (h d)"),
                        t_all[:, 1, :nch].rearrange("p c h d -> p c (h d)"))
                    tile_epilogue(t, Wq, nch, slab)

            # ---- phase B/M: 3-segment tiles (both-regions + kinded mixed) --
            # kinds per mixed tile chunk: 'g' (all-global: vp slot 3 for T0,
            # coef1 zeroed), 'n' (all non-global: batched recips are exact),
            # 'x' (mixed a0: am-driven coefficient assembly)
            with (
                tc.tile_pool(name="bsc", bufs=2, space="PSUM") as bsc,
                tc.tile_pool(name="bT", bufs=1, space="PSUM") as bT,
            ):
                t0 = t_g + t_r1 + t_r2
                for t in range(t0, N_TILES):
                    tk = kinds[t - t0 - t_b] if t >= t0 + t_b else None
                    Wq, nch, qt_t = tile_prologue(t)
                    slab = slab_pool.tile([128, 4, H * D], bf16)
                    t_all = tall_pool.tile([128, 2, 4, H, 64], bf16)

                    def b_scores(h):
                        sc = bsc.tile([128, 3, W_TILE], f32, tag="bs")
                        for r in range(3):
                            nc.tensor.matmul(
                                sc[:, r, :Wq], lhsT=kt_sb[:, r, h, :],
                                rhs=qt_t[:, h, :Wq], start=True, stop=True)
                        return sc

                    sc_h = b_scores(0)
                    for h in range(H):
                        e = e_pool.tile([128, 3, W_TILE], bf16, tag="e3")
                        nc.scalar.activation(e[:, :, :Wq], sc_h[:, :, :Wq],
                                             Exp)
                        if h + 1 < H:
                            sc_h = b_scores(h + 1)
                        T = bT.tile([128, 2, W_TILE], f32, tag="T")
                        for c in range(nch):
                            cs = slice(c * 128, (c + 1) * 128)
                            o = slice(65 * c, 65 * c + 65)
                            slot0 = 3 if (tk and tk[c] == 'g') else 0
                            nc.tensor.matmul(T[:, 0, o], lhsT=e[:, 0, cs],
                                             rhs=vp_sb[:, slot0, h, :],
                                             start=True, stop=True)
                            nc.tensor.matmul(T[:, 1, o], lhsT=e[:, 1, cs],
                                             rhs=vp_sb[:, 1, h, :],
                                             start=True, stop=False)
                            nc.tensor.matmul(T[:, 1, o], lhsT=e[:, 2, cs],
                                             rhs=vp_sb[:, 2, h, :],
                                             start=False, stop=True)
                        tn, td = t_views(T, nch)
                        coef = sm_pool.tile([128, 2, 4], f32, tag="ri2")
                        # batched: coef0 = 1/T0d (all chunks), coef1 =
                        # 1/T12d (exact for 'n'; 'g' overwritten by memset,
                        # 'x' overwritten below)
                        nc.vector.reciprocal(coef[:, :, :nch], td)
                        if tk:
                            for c in range(nch):
                                if tk[c] == 'g':
                                    nc.gpsimd.memset(coef[:, 1, c:c + 1],
                                                     0.0)
                                elif tk[c] == 'x':
                                    a0 = am_sb[:, t, c:c + 1]
                                    m0 = sm_pool.tile([128, 1], f32,
                                                      tag="sm")
                                    nc.vector.tensor_mul(m0, a0,
                                                         td[:, 0, c:c + 1])
                                    wd = sm_pool.tile([128, 1], f32,
                                                      tag="sm")
                                    nc.vector.tensor_add(wd, m0,
                                                         td[:, 1, c:c + 1])
                                    nc.vector.reciprocal(
                                        coef[:, 1, c:c + 1], wd)
                                    c0a = sm_pool.tile([128, 1], f32,
                                                       tag="sm")
                                    nc.vector.tensor_mul(
                                        c0a, a0, coef[:, 1, c:c + 1])
                                    nc.vector.tensor_add(
                                        coef[:, 0, c:c + 1], c0a,
                                        coef[:, 0, c:c + 1])
                        nc.vector.tensor_mul(
                            t_all[:, :, :nch, h, :], tn,
                            coef[:, :, :nch, None].broadcast_to(
                                [128, 2, nch, 64]))
                    nc.gpsimd.tensor_add(
                        slab[:, :nch, :],
                        t_all[:, 0, :nch].rearrange("p c h d -> p c (h d)"),
                        t_all[:, 1, :nch].rearrange("p c h d -> p c (h d)"))
                    tile_epilogue(t, Wq, nch, slab)

    nc.compile()
    return nc


def _get_compiled(gt):
    if gt not in _COMPILED:
        _COMPILED[gt] = _build_kernel(gt)
    return _COMPILED[gt]


# ---------------------------------------------------------------- frontend
def _prepare(q, k, v, regional_k, regional_v, region_masks):
    bf = ml_dtypes.bfloat16
    q = np.asarray(q, dtype=np.float32)
    k = np.asarray(k, dtype=np.float32)
    v = np.asarray(v, dtype=np.float32)
    regional_k = np.asarray(regional_k, dtype=np.float32)
    regional_v = np.asarray(regional_v, dtype=np.float32)
    region_masks = np.asarray(region_masks, dtype=np.float32)

    a0, a1, a2 = _preprocess_masks(region_masks)  # [S] each

    # 5-way category sort: global-only / r1-only / r2-only / both / mixed.
    # Each core gets identical leading tile counts per category (SPMD needs
    # one graph); leftovers fall back to the mixed path which is correct
    # for any query.
    cats = [
        np.nonzero(a0 == 1.0)[0],
        np.nonzero((a1 == 1.0) & (a2 == 0.0))[0],
        np.nonzero((a2 == 1.0) & (a1 == 0.0))[0],
        np.nonzero((a1 == 1.0) & (a2 == 1.0))[0],
    ]
    counts = []
    used_parts = []
    leftover_parts = []
    budget = N_TILES - 1  # keep at least one mixed tile (incl. ragged tail)
    for idx in cats:
        tcnt = min(len(idx) // (N_CORES * W_TILE), budget)
        budget -= tcnt
        counts.append(tcnt)
        n_used = tcnt * W_TILE * N_CORES
        used_parts.append(idx[:n_used])
        leftover_parts.append(idx[n_used:])
    t_g, t_r1, t_r2, t_b = counts
    ns = [c * W_TILE for c in counts]
    n_left = SSH - sum(ns)

    # mixed region: balance leftover categories across cores so every core
    # holds exactly n_left queries with an identical chunk-kind pattern
    # (global-first). Remainders are distributed one per core.
    base = [len(p) // N_CORES for p in leftover_parts]
    rems = [len(p) % N_CORES for p in leftover_parts]
    deficit = n_left - sum(base)   # how many +1s each core needs
    take = np.array([base] * N_CORES)           # [core, cat]
    pool_idx = [(ci, None) for ci in range(4) for _ in range(rems[ci])]
    assert len(pool_idx) == deficit * N_CORES, (rems, deficit)
    for core in range(N_CORES):
        for _ in range(deficit):
            ci, _n = pool_idx.pop()
            take[core, ci] += 1
    offs = [0, 0, 0, 0]
    core_mixed = []
    for core in range(N_CORES):
        parts = []
        for ci in range(4):
            n = take[core, ci]
            parts.append(leftover_parts[ci][offs[ci]:offs[ci] + n])
            offs[ci] += n
        core_mixed.append(np.concatenate(parts))
    # chunk kinds from per-core g-prefix lengths (identical graphs needed)
    n_mix_tiles = N_TILES - sum(counts)
    kinds = []
    for mt in range(n_mix_tiles):
        tile_kinds = []
        s0 = mt * W_TILE
        wq = min(W_TILE, n_left - s0)
        for c in range(wq // 128):
            ks = set()
            for core in range(N_CORES):
                glen = int(take[core, 0])
                clo, chi = s0 + c * 128, s0 + c * 128 + 128
                if chi <= glen:
                    ks.add('g')
                elif clo >= glen:
                    ks.add('n')
                else:
                    ks.add('x')
            tile_kinds.append('x' if len(ks) > 1 else ks.pop())
        kinds.append(tuple(tile_kinds))
    kinds = tuple(kinds)

    perm = np.empty(S, dtype=np.int64)
    for c in range(N_CORES):
        lo = c * SSH
        off = 0
        for ncat, part in zip(ns, used_parts):
            perm[lo + off:lo + off + ncat] = part[c * ncat:(c + 1) * ncat]
            off += ncat
        perm[lo + off:lo + SSH] = core_mixed[c]
    gt = (t_g, t_r1, t_r2, t_b, kinds)

    a0p = a0[perm]
    b1 = (NEG_BIAS * (1.0 - a1[perm])).astype(bf)
    b2 = (NEG_BIAS * (1.0 - a2[perm])).astype(bf)

    # qT plus bias rows: [66, S, H]
    qt66 = np.zeros((KQ, S, H), dtype=bf)
    qt66[:64] = q[0][perm].transpose(2, 0, 1).astype(bf)
    qt66[64] = b1[:, None]
    qt66[65] = b2[:, None]

    # kT*scale plus selector rows: [66, 3, H, P] bf16
    k_segs = np.stack([k[0], regional_k[0, 0], regional_k[1, 0]], axis=0)
    kt = np.zeros((3, H, KQ, P), dtype=np.float32)
    kt[:, :, :64, :] = k_segs.transpose(0, 2, 3, 1) * np.float32(SCALE)
    kt[1, :, 64, :] = 1.0
    kt[2, :, 65, :] = 1.0
    kt = np.ascontiguousarray(kt.astype(bf).transpose(2, 0, 1, 3))

    # V plus denominator column: [128, 4, H, 65] bf16
    # slot 0: v0 with 2.0 col (0.5-blend baked), 1/2: regional, 3: v0 with
    # 1.0 col (global tiles)
    v_segs = np.stack([v[0], regional_v[0, 0], regional_v[1, 0], v[0]],
                      axis=0)
    vp = np.empty((4, H, P, 65), dtype=np.float32)
    vp[..., :64] = v_segs.transpose(0, 2, 1, 3)
    vp[..., 64] = 2.0
    vp[3, ..., 64] = 1.0
    vp = np.ascontiguousarray(vp.astype(bf).transpose(2, 0, 1, 3))

    in_maps = []
    pad = N_TILES * W_TILE - SSH
    for core in range(N_CORES):
        lo = core * SSH
        am = np.zeros((N_TILES, 128, 4), np.float32)
        for t in range(sum(gt[:4]), N_TILES):
            s0 = t * W_TILE
            Wq = min(W_TILE, SSH - s0)
            nch = Wq // 128
            am[t, :, :nch] = a0p[lo + s0: lo + s0 + Wq].reshape(nch, 128).T
        qtc = qt66[:, lo:lo + SSH, :]                       # [66, SSH, H]
        qtc = np.concatenate(
            [qtc, np.zeros((KQ, pad, H), dtype=bf)], axis=1)
        qtc = qtc.reshape(KQ, N_TILES, W_TILE, H).transpose(0, 1, 3, 2)
        in_maps.append({
            "qt": np.ascontiguousarray(qtc),                # [66,NT,H,W]
            "kt": kt,
            "vp": vp,
            "am": np.ascontiguousarray(am.transpose(1, 0, 2)),  # [128,NT,4]
        })
    return in_maps, perm, gt


def kernel(q, k, v, regional_k, regional_v, region_masks):
    from concourse.bass_utils import run_bass_kernel_spmd

    in_maps, perm, gt = _prepare(q, k, v, regional_k, regional_v,
                                 region_masks)
    nc = _get_compiled(gt)
    res = run_bass_kernel_spmd(nc, in_maps, core_ids=list(range(N_CORES)))
    out_sorted = np.concatenate(
        [np.asarray(res.results[i]["out"]).astype(np.float32)
         for i in range(N_CORES)], axis=0)
    out = np.empty_like(out_sorted)
    out[perm] = out_sorted
    return out.reshape(1, S, H * D).astype(np.float32)


# revision 6
# speedup vs baseline: 1.1880x; 1.1880x over previous
"""Sparse regional cross-attention on 8 Trainium2 NeuronCores.

Reference computation (B=1, S=56320, H=8, D=64, P=128, R=2):
  - per-region binary masks over the latent sequence select which KV segments
    each query may attend to (global prompt + R regional prompts, 128 keys
    each); regional pass = softmax over the union of allowed segments
  - base pass: plain softmax attention over the global prompt
  - out = 0.5 * regional + 0.5 * base

Kernel strategy (v2):
  - sequence-parallel across 8 cores (7040 queries/core); queries host-sorted
    into 5 tile categories: global-only (segs {0}), region-1-only ({0,1}),
    region-2-only ({0,2}), both-regions ({0,1,2}, a0=0), mixed leftovers
    ({0,1,2} with per-chunk a0 data). Output un-permuted on host.
  - per (head, 512-query tile): K=66 bf16 score matmuls (qT + 2 bias rows;
    measured: K=66 streams at full rate), ONE fused exp per head over all
    segments, chunked PV matmuls (N=65, FWL-bound ~55ns) into a combined
    [128, 2, 512] PSUM tile: T0-half (base, vp has 2.0 denom col) and
    T12-half (regional segs accumulated); chunk c at offset 65c avoids bank
    crossings.
  - merge per head: ONE batched reciprocal [128,2,4] over both denominator
    sets + ONE fused tensor_mul over [128,2,4,64] numerators with the
    reciprocal broadcast; per tile a single gpsimd add folds the two halves
    into the output slab. The 0.5 blend is baked into vp's 2.0 denominator
    column (global tiles use a 1.0 column).
"""

import sys

for _p in ("/opt/trn_rl_repo",):
    if _p not in sys.path:
        sys.path.insert(0, _p)

import numpy as np
import ml_dtypes

# ---------------------------------------------------------------- constants
B, S, H, D, P, R = 1, 56320, 8, 64, 128, 2
N_CORES = 8
SSH = S // N_CORES          # 7040 queries per core
W_TILE = 512                # queries per tile
N_TILES = (SSH + W_TILE - 1) // W_TILE   # 14 (13 full + 1x384)
KQ = 66                     # q rows: 64 dims + 2 bias rows
LAT_T, LAT_H, LAT_W = 16, 44, 80
SCALE = D ** -0.5
NEG_BIAS = -30.0

_COMPILED = {}


# ------------------------------------------------------------ mask pipeline
def _resize_trilinear_np(m, tgt_shape):
    """numpy replica of jax.image.resize(..., 'trilinear', antialias=False)."""
    Bn, C, T, Hh, Ww = m.shape
    _, _, tT, tH, tW = tgt_shape
    out = m.astype(np.float32)

    def lin_weights(n_in, n_out):
        j = np.arange(n_out, dtype=np.float64)
        x = (j + 0.5) * (n_in / n_out) - 0.5
        lo = np.floor(x).astype(np.int64)
        frac = (x - lo).astype(np.float32)
        lo0 = np.clip(lo, 0, n_in - 1)
        lo1 = np.clip(lo + 1, 0, n_in - 1)
        Wm = np.zeros((n_out, n_in), np.float32)
        Wm[np.arange(n_out), lo0] += 1.0 - frac
        Wm[np.arange(n_out), lo1] += frac
        return Wm

    out = np.einsum('oi,bcihw->bcohw', lin_weights(T, tT), out)
    out = np.einsum('oi,bctiw->bctow', lin_weights(Hh, tH), out)
    out = np.einsum('oi,bcthi->bctho', lin_weights(Ww, tW), out)
    return out.astype(np.float32)


def _preprocess_mask_np(mask):
    m = np.transpose(mask, (3, 0, 1, 2))[:, None]  # [B,1,T,H,W]
    Bn = m.shape[0]
    T = m.shape[2]
    tgt = (Bn, 1, 1, LAT_H, LAT_W)
    pieces = [_resize_trilinear_np(m[:, :, :1], tgt)]
    for wi in range(1, T, 8):
        pieces.append(_resize_trilinear_np(m[:, :, wi:wi + 8], tgt))
    mm = np.concatenate(pieces, axis=2)[:, 0]
    return (mm > 0.5).astype(np.float32).reshape(Bn, -1)


def _preprocess_masks(region_masks):
    """region_masks [R, T, MH, MW, B] -> a0, a1, a2 each [S] float32 {0,1}."""
    try:
        import jax
        import jax.numpy as jnp

        cpu = jax.devices('cpu')[0]
        with jax.default_device(cpu):
            def one(mask):
                m = jnp.transpose(jnp.asarray(mask), (3, 0, 1, 2))[:, None]
                Bn, _, T, _, _ = m.shape
                tgt = (Bn, 1, 1, LAT_H, LAT_W)
                pieces = [jax.image.resize(m[:, :, :1], tgt, 'trilinear',
                                           antialias=False)]
                for wi in range(1, T, 8):
                    pieces.append(jax.image.resize(m[:, :, wi:wi + 8], tgt,
                                                   'trilinear',
                                                   antialias=False))
                mm = jnp.concatenate(pieces, axis=2)[:, 0]
                return (mm > 0.5).astype(jnp.float32).reshape(Bn, -1)

            masks = np.stack([np.asarray(one(region_masks[i]))
                              for i in range(region_masks.shape[0])], axis=0)
    except Exception:
        masks = np.stack([_preprocess_mask_np(region_masks[i])
                          for i in range(region_masks.shape[0])], axis=0)
    a1 = masks[0, 0]
    a2 = masks[1, 0]
    a0 = ((masks[0, 0] + masks[1, 0]) == 0).astype(np.float32)
    return a0, a1, a2


# ------------------------------------------------------------- bass kernel
def _build_kernel(cfg):
    """cfg = (t_g, t_r1, t_r2, t_b, kinds): leading tile counts per core for
    the global-only / region-1-only / region-2-only / both-regions
    categories; the rest are mixed 3-segment tiles whose per-chunk kinds
    ('g'/'n'/'x') are compile-time constants."""
    import concourse.bass as bass
    import concourse.tile as tile
    from concourse import bacc, mybir

    f32 = mybir.dt.float32
    bf16 = mybir.dt.bfloat16
    Exp = mybir.ActivationFunctionType.Exp

    t_g, t_r1, t_r2, t_b, kinds = cfg

    nc = bacc.Bacc("TRN2", target_bir_lowering=False, debug=False,
                   num_devices=N_CORES)

    qt_d = nc.dram_tensor("qt", [KQ, N_TILES, H, W_TILE], bf16,
                          kind="ExternalInput").ap()
    kt_d = nc.dram_tensor("kt", [KQ, 3, H, P], bf16,
                          kind="ExternalInput").ap()
    vp_d = nc.dram_tensor("vp", [128, 4, H, 65], bf16,
                          kind="ExternalInput").ap()
    am_d = nc.dram_tensor("am", [128, N_TILES, 4], f32,
                          kind="ExternalInput").ap()
    out_d = nc.dram_tensor("out", [SSH, H * D], bf16,
                           kind="ExternalOutput").ap()

    with tile.TileContext(nc) as tc:
        with (
            tc.tile_pool(name="singles", bufs=1) as singles,
            tc.tile_pool(name="qt", bufs=2) as qt_pool,
            tc.tile_pool(name="epool", bufs=3) as e_pool,
            tc.tile_pool(name="small", bufs=16) as sm_pool,
            tc.tile_pool(name="tall", bufs=2) as tall_pool,
            tc.tile_pool(name="slab", bufs=2) as slab_pool,
        ):
            kt_sb = singles.tile([KQ, 3, H, P], bf16)
            nc.sync.dma_start(out=kt_sb, in_=kt_d)
            vp_sb = singles.tile([128, 4, H, 65], bf16)
            nc.sync.dma_start(out=vp_sb, in_=vp_d)
            am_sb = singles.tile([128, N_TILES, 4], f32)
            nc.sync.dma_start(out=am_sb, in_=am_d)

            # PE warm-up: ~5us of dummy matmuls off a memset tile so the
            # HAM clock-gate opens before real work and the PE never sits
            # idle during the input DMAs (a cold-stuck PE halves matmul
            # throughput for the whole kernel).
            with tc.tile_pool(name="warm", bufs=1, space="PSUM") as wp:
                wtile = singles.tile([128, 512], bf16)
                nc.vector.memset(wtile, 0.0)
                wps = wp.tile([128, 512], f32)
                for _ in range(24):
                    nc.tensor.matmul(wps, lhsT=wtile[:, :128],
                                     rhs=wtile, start=True, stop=True)

            def tile_prologue(t):
                Wq = min(W_TILE, SSH - t * W_TILE)
                nch = Wq // 128
                qt_t = qt_pool.tile([KQ, H, W_TILE], bf16)
                nc.sync.dma_start(out=qt_t, in_=qt_d[:, t])
                return Wq, nch, qt_t

            def tile_epilogue(t, Wq, nch, slab):
                s0 = t * W_TILE
                nc.sync.dma_start(
                    out=out_d[s0:s0 + Wq, :].rearrange("(c p) f -> p c f",
                                                       p=128),
                    in_=slab[:, :nch, :])

            def t_views(T, nch):
                """numerator [128,2,nch,64] and denominator [128,2,nch]
                views of a combined [128, 2, 512] PSUM tile with chunk c
                at offset 65c."""
                vv = T[:, :, :260].rearrange("p w (c x) -> p w c x", x=65)
                return vv[:, :, :nch, 0:64], vv[:, :, :nch, 64]

            # ---- phase G: global-only tiles (segment 0; vp slot 3) ----
            with (
                tc.tile_pool(name="gsc", bufs=2, space="PSUM") as gsc,
                tc.tile_pool(name="gt0", bufs=2, space="PSUM") as gt0,
            ):
                for t in range(t_g):
                    Wq, nch, qt_t = tile_prologue(t)
                    slab = slab_pool.tile([128, 4, H * D], bf16)

                    def g_scores(h):
                        sc = gsc.tile([128, W_TILE], f32, tag="gs")
                        nc.tensor.matmul(
                            sc[:, :Wq], lhsT=kt_sb[:, 0, h, :],
                            rhs=qt_t[:, h, :Wq], start=True, stop=True)
                        return sc

                    sc_h = g_scores(0)
                    for h in range(H):
                        e = e_pool.tile([128, W_TILE], bf16, tag="e1")
                        nc.scalar.activation(e[:, :Wq], sc_h[:, :Wq], Exp)
                        if h + 1 < H:
                            sc_h = g_scores(h + 1)
                        T = gt0.tile([128, 4, 65], f32, tag="T")
                        for c in range(nch):
                            cs = slice(c * 128, (c + 1) * 128)
                            nc.tensor.matmul(T[:, c, :], lhsT=e[:, cs],
                                             rhs=vp_sb[:, 3, h, :],
                                             start=True, stop=True)
                        ri = sm_pool.tile([128, 4], f32, tag="ri")
                        nc.vector.reciprocal(ri[:, :nch], T[:, :nch, 64])
                        nc.vector.tensor_mul(
                            slab[:, :nch, h * 64:(h + 1) * 64],
                            T[:, :nch, 0:64],
                            ri[:, :nch, None].broadcast_to([128, nch, 64]))
                    tile_epilogue(t, Wq, nch, slab)

            # ---- phase R: single-region tiles (segs {0, r}) ----
            with (
                tc.tile_pool(name="rsc", bufs=2, space="PSUM") as rsc,
                tc.tile_pool(name="rT", bufs=2, space="PSUM") as rT,
            ):
                for t in range(t_g, t_g + t_r1 + t_r2):
                    rseg = 1 if t < t_g + t_r1 else 2
                    Wq, nch, qt_t = tile_prologue(t)
                    slab = slab_pool.tile([128, 4, H * D], bf16)
                    t_all = tall_pool.tile([128, 2, 4, H, 64], bf16)

                    def r_scores(h):
                        sc = rsc.tile([128, 2, W_TILE], f32, tag="rs")
                        for j, r in enumerate((0, rseg)):
                            nc.tensor.matmul(
                                sc[:, j, :Wq], lhsT=kt_sb[:, r, h, :],
                                rhs=qt_t[:, h, :Wq], start=True, stop=True)
                        return sc

                    sc_h = r_scores(0)
                    for h in range(H):
                        e = e_pool.tile([128, 2, W_TILE], bf16, tag="e2")
                        nc.scalar.activation(e[:, :, :Wq], sc_h[:, :, :Wq],
                                             Exp)
                        if h + 1 < H:
                            sc_h = r_scores(h + 1)
                        T = rT.tile([128, 2, W_TILE], f32, tag="T")
                        for c in range(nch):
                            cs = slice(c * 128, (c + 1) * 128)
                            o = slice(65 * c, 65 * c + 65)
                            nc.tensor.matmul(T[:, 0, o], lhsT=e[:, 0, cs],
                                             rhs=vp_sb[:, 0, h, :],
                                             start=True, stop=True)
                            nc.tensor.matmul(T[:, 1, o], lhsT=e[:, 1, cs],
                                             rhs=vp_sb[:, rseg, h, :],
                                             start=True, stop=True)
                        tn, td = t_views(T, nch)
                        ri = sm_pool.tile([128, 2, 4], f32, tag="ri2")
                        nc.vector.reciprocal(ri[:, :, :nch], td)
                        nc.vector.tensor_mul(
                            t_all[:, :, :nch, h, :], tn,
                            ri[:, :, :nch, None].broadcast_to(
                                [128, 2, nch, 64]))
                    nc.gpsimd.tensor_add(
                        slab[:, :nch, :],
                        t_all[:, 0, :nch].rearrange("p c h d -> p c (h d)"),
                        t_all[:, 1, :nch].rearrange("p c h d -> p c (h d)"))
                    tile_epilogue(t, Wq, nch, slab)

            # ---- phase B/M: 3-segment tiles (both-regions + kinded mixed) --
            # kinds per mixed tile chunk: 'g' (all-global: vp slot 3 for T0,
            # coef1 zeroed), 'n' (all non-global: batched recips are exact),
            # 'x' (mixed a0: am-driven coefficient assembly)
            with (
                tc.tile_pool(name="bsc", bufs=2, space="PSUM") as bsc,
                tc.tile_pool(name="bT", bufs=1, space="PSUM") as bT,
            ):
                t0 = t_g + t_r1 + t_r2
                for t in range(t0, N_TILES):
                    tk = kinds[t - t0 - t_b] if t >= t0 + t_b else None
                    Wq, nch, qt_t = tile_prologue(t)
                    slab = slab_pool.tile([128, 4, H * D], bf16)
                    t_all = tall_pool.tile([128, 2, 4, H, 64], bf16)

                    def b_scores(h):
                        sc = bsc.tile([128, 3, W_TILE], f32, tag="bs")
                        for r in range(3):
                            nc.tensor.matmul(
                                sc[:, r, :Wq], lhsT=kt_sb[:, r, h, :],
                                rhs=qt_t[:, h, :Wq], start=True, stop=True)
                        return sc

                    sc_h = b_scores(0)
                    for h in range(H):
                        e = e_pool.tile([128, 3, W_TILE], bf16, tag="e3")
                        nc.scalar.activation(e[:, :, :Wq], sc_h[:, :, :Wq],
                                             Exp)
                        if h + 1 < H:
                            sc_h = b_scores(h + 1)
                        T = bT.tile([128, 2, W_TILE], f32, tag="T")
                        for c in range(nch):
                            cs = slice(c * 128, (c + 1) * 128)
                            o = slice(65 * c, 65 * c + 65)
                            slot0 = 3 if (tk and tk[c] == 'g') else 0
                            nc.tensor.matmul(T[:, 0, o], lhsT=e[:, 0, cs],
                                             rhs=vp_sb[:, slot0, h, :],
                                             start=True, stop=True)
                            nc.tensor.matmul(T[:, 1, o], lhsT=e[:, 1, cs],
                                             rhs=vp_sb[:, 1, h, :],
                                             start=True, stop=False)
                            nc.tensor.matmul(T[:, 1, o], lhsT=e[:, 2, cs],
                                             rhs=vp_sb[:, 2, h, :],
                                             start=False, stop=True)
                        tn, td = t_views(T, nch)
                        coef = sm_pool.tile([128, 2, 4], f32, tag="ri2")
                        # batched: coef0 = 1/T0d (all chunks), coef1 =
                        # 1/T12d (exact for 'n'; 'g' overwritten by memset,
                        # 'x' overwritten below)
                        nc.vector.reciprocal(coef[:, :, :nch], td)
                        if tk:
                            for c in range(nch):
                                if tk[c] == 'g':
                                    nc.gpsimd.memset(coef[:, 1, c:c + 1],
                                                     0.0)
                                elif tk[c] == 'x':
                                    a0 = am_sb[:, t, c:c + 1]
                                    m0 = sm_pool.tile([128, 1], f32,
                                                      tag="sm")
                                    nc.vector.tensor_mul(m0, a0,
                                                         td[:, 0, c:c + 1])
                                    wd = sm_pool.tile([128, 1], f32,
                                                      tag="sm")
                                    nc.vector.tensor_add(wd, m0,
                                                         td[:, 1, c:c + 1])
                                    nc.vector.reciprocal(
                                        coef[:, 1, c:c + 1], wd)
                                    c0a = sm_pool.tile([128, 1], f32,
                                                       tag="sm")
                                    nc.vector.tensor_mul(
                                        c0a, a0, coef[:, 1, c:c + 1])
                                    nc.vector.tensor_add(
                                        coef[:, 0, c:c + 1], c0a,
                                        coef[:, 0, c:c + 1])
                        nc.vector.tensor_mul(
                            t_all[:, :, :nch, h, :], tn,
                            coef[:, :, :nch, None].broadcast_to(
                                [128, 2, nch, 64]))
                    nc.gpsimd.tensor_add(
                        slab[:, :nch, :],
                        t_all[:, 0, :nch].rearrange("p c h d -> p c (h d)"),
                        t_all[:, 1, :nch].rearrange("p c h d -> p c (h d)"))
                    tile_epilogue(t, Wq, nch, slab)

    nc.compile()
    return nc


def _get_compiled(gt):
    if gt not in _COMPILED:
        _COMPILED[gt] = _build_kernel(gt)
    return _COMPILED[gt]


# ---------------------------------------------------------------- frontend
def _prepare(q, k, v, regional_k, regional_v, region_masks):
    bf = ml_dtypes.bfloat16
    q = np.asarray(q, dtype=np.float32)
    k = np.asarray(k, dtype=np.float32)
    v = np.asarray(v, dtype=np.float32)
    regional_k = np.asarray(regional_k, dtype=np.float32)
    regional_v = np.asarray(regional_v, dtype=np.float32)
    region_masks = np.asarray(region_masks, dtype=np.float32)

    a0, a1, a2 = _preprocess_masks(region_masks)  # [S] each

    # 5-way category sort: global-only / r1-only / r2-only / both / mixed.
    # Each core gets identical leading tile counts per category (SPMD needs
    # one graph); leftovers fall back to the mixed path which is correct
    # for any query.
    cats = [
        np.nonzero(a0 == 1.0)[0],
        np.nonzero((a1 == 1.0) & (a2 == 0.0))[0],
        np.nonzero((a2 == 1.0) & (a1 == 0.0))[0],
        np.nonzero((a1 == 1.0) & (a2 == 1.0))[0],
    ]
    counts = []
    used_parts = []
    leftover_parts = []
    budget = N_TILES - 1  # keep at least one mixed tile (incl. ragged tail)
    for idx in cats:
        tcnt = min(len(idx) // (N_CORES * W_TILE), budget)
        budget -= tcnt
        counts.append(tcnt)
        n_used = tcnt * W_TILE * N_CORES
        used_parts.append(idx[:n_used])
        leftover_parts.append(idx[n_used:])
    t_g, t_r1, t_r2, t_b = counts
    ns = [c * W_TILE for c in counts]
    n_left = SSH - sum(ns)

    # mixed region: balance leftover categories across cores so every core
    # holds exactly n_left queries with an identical chunk-kind pattern
    # (global-first). Remainders are distributed one per core.
    base = [len(p) // N_CORES for p in leftover_parts]
    rems = [len(p) % N_CORES for p in leftover_parts]
    deficit = n_left - sum(base)   # how many +1s each core needs
    take = np.array([base] * N_CORES)           # [core, cat]
    pool_idx = [(ci, None) for ci in range(4) for _ in range(rems[ci])]
    assert len(pool_idx) == deficit * N_CORES, (rems, deficit)
    for core in range(N_CORES):
        for _ in range(deficit):
            ci, _n = pool_idx.pop()
            take[core, ci] += 1
    offs = [0, 0, 0, 0]
    core_mixed = []
    for core in range(N_CORES):
        parts = []
        for ci in range(4):
            n = take[core, ci]
            parts.append(leftover_parts[ci][offs[ci]:offs[ci] + n])
            offs[ci] += n
        core_mixed.append(np.concatenate(parts))
    # chunk kinds from per-core g-prefix lengths (identical graphs needed)
    n_mix_tiles = N_TILES - sum(counts)
    kinds = []
    for mt in range(n_mix_tiles):
        tile_kinds = []
        s0 = mt * W_TILE
        wq = min(W_TILE, n_left - s0)
        for c in range(wq // 128):
            ks = set()
            for core in range(N_CORES):
                glen = int(take[core, 0])
                clo, chi = s0 + c * 128, s0 + c * 128 + 128
                if chi <= glen:
                    ks.add('g')
                elif clo >= glen:
                    ks.add('n')
                else:
                    ks.add('x')
            tile_kinds.append('x' if len(ks) > 1 else ks.pop())
        kinds.append(tuple(tile_kinds))
    kinds = tuple(kinds)

    perm = np.empty(S, dtype=np.int64)
    for c in range(N_CORES):
        lo = c * SSH
        off = 0
        for ncat, part in zip(ns, used_parts):
            perm[lo + off:lo + off + ncat] = part[c * ncat:(c + 1) * ncat]
            off += ncat
        perm[lo + off:lo + SSH] = core_mixed[c]
    gt = (t_g, t_r1, t_r2, t_b, kinds)

    a0p = a0[perm]
    b1 = (NEG_BIAS * (1.0 - a1[perm])).astype(bf)
    b2 = (NEG_BIAS * (1.0 - a2[perm])).astype(bf)

    # qT plus bias rows: [66, S, H]
    qt66 = np.zeros((KQ, S, H), dtype=bf)
    qt66[:64] = q[0][perm].transpose(2, 0, 1).astype(bf)
    qt66[64] = b1[:, None]
    qt66[65] = b2[:, None]

    # kT*scale plus selector rows: [66, 3, H, P] bf16
    k_segs = np.stack([k[0], regional_k[0, 0], regional_k[1, 0]], axis=0)
    kt = np.zeros((3, H, KQ, P), dtype=np.float32)
    kt[:, :, :64, :] = k_segs.transpose(0, 2, 3, 1) * np.float32(SCALE)
    kt[1, :, 64, :] = 1.0
    kt[2, :, 65, :] = 1.0
    kt = np.ascontiguousarray(kt.astype(bf).transpose(2, 0, 1, 3))

    # V plus denominator column: [128, 4, H, 65] bf16
    # slot 0: v0 with 2.0 col (0.5-blend baked), 1/2: regional, 3: v0 with
    # 1.0 col (global tiles)
    v_segs = np.stack([v[0], regional_v[0, 0], regional_v[1, 0], v[0]],
                      axis=0)
    vp = np.empty((4, H, P, 65), dtype=np.float32)
    vp[..., :64] = v_segs.transpose(0, 2, 1, 3)
    vp[..., 64] = 2.0
    vp[3, ..., 64] = 1.0
    vp = np.ascontiguousarray(vp.astype(bf).transpose(2, 0, 1, 3))

    in_maps = []
    pad = N_TILES * W_TILE - SSH
    for core in range(N_CORES):
        lo = core * SSH
        am = np.zeros((N_TILES, 128, 4), np.float32)
        for t in range(sum(gt[:4]), N_TILES):
            s0 = t * W_TILE
            Wq = min(W_TILE, SSH - s0)
            nch = Wq // 128
            am[t, :, :nch] = a0p[lo + s0: lo + s0 + Wq].reshape(nch, 128).T
        qtc = qt66[:, lo:lo + SSH, :]                       # [66, SSH, H]
        qtc = np.concatenate(
            [qtc, np.zeros((KQ, pad, H), dtype=bf)], axis=1)
        qtc = qtc.reshape(KQ, N_TILES, W_TILE, H).transpose(0, 1, 3, 2)
        in_maps.append({
            "qt": np.ascontiguousarray(qtc),                # [66,NT,H,W]
            "kt": kt,
            "vp": vp,
            "am": np.ascontiguousarray(am.transpose(1, 0, 2)),  # [128,NT,4]
        })
    return in_maps, perm, gt


def kernel(q, k, v, regional_k, regional_v, region_masks):
    from concourse.bass_utils import run_bass_kernel_spmd

    in_maps, perm, gt = _prepare(q, k, v, regional_k, regional_v,
                                 region_masks)
    nc = _get_compiled(gt)
    res = run_bass_kernel_spmd(nc, in_maps, core_ids=list(range(N_CORES)))
    out_sorted = np.concatenate(
        [np.asarray(res.results[i]["out"]).astype(np.float32)
         for i in range(N_CORES)], axis=0)
    out = np.empty_like(out_sorted)
    out[perm] = out_sorted
    return out.reshape(1, S, H * D).astype(np.float32)
